# revision 5
# baseline (speedup 1.0000x reference)
"""Multi-head attention (RoPE + causal-mask softmax) on 8 TRN2 NeuronCores.

Sharding: batch x q-chunk (2 batches x 4 chunks of 512 query rows). Each core
computes all 16 heads for its 512 query rows. K/V are recomputed per core
(whole batch), so there are no collectives; outputs are disjoint slices.

To make the program SPMD-uniform, each core's sequence axis is rolled so its
query window sits at s=0 (attention is permutation-invariant over the key
axis when K, V, mask and the RoPE tables are rolled together).
"""

from contextlib import ExitStack

import numpy as np

import concourse.bass as bass
import concourse.tile as tile
from concourse import bacc, mybir
from concourse.alu_op_type import AluOpType
from concourse.bass_utils import run_bass_kernel_spmd

AF = mybir.ActivationFunctionType
F32 = mybir.dt.float32
F32R = mybir.dt.float32r
F16 = mybir.dt.float16

B, S, HID, NH, HD = 2, 2048, 1024, 16, 64
SCALE = 1.0 / np.sqrt(HD)
N_CORES = 8
Q = 512          # query rows per core
HC = HID // 128  # hidden chunks (8)
PAIRS = NH // 2  # head pairs (8)
SC16 = S // 128  # key chunks of 128 (16)
SC4 = S // 512   # key chunks of 512 (4)


def build_program(dbg: bool = False):
    nc = bacc.Bacc("TRN2", target_bir_lowering=False, debug=False,
                   num_devices=N_CORES)

    hsT = nc.dram_tensor("hsT", [HID, S], F32R, kind="ExternalInput").ap()
    cosk = nc.dram_tensor("cosk", [128, S], F32, kind="ExternalInput").ap()
    sink = nc.dram_tensor("sink", [128, S], F32, kind="ExternalInput").ap()
    emask = nc.dram_tensor("emask", [S, Q], F16, kind="ExternalInput").ap()
    wq = nc.dram_tensor("wq", [HID, HID], F32R, kind="ExternalInput").ap()
    wk = nc.dram_tensor("wk", [HID, HID], F32R, kind="ExternalInput").ap()
    wv = nc.dram_tensor("wv", [HID, HID], F32R, kind="ExternalInput").ap()
    wo = nc.dram_tensor("wo", [HID, HID], F16, kind="ExternalInput").ap()
    out = nc.dram_tensor("out", [Q, HID], F32, kind="ExternalOutput").ap()
    if dbg:
        d_kt = nc.dram_tensor("d_kt", [128, S], F32, kind="ExternalOutput").ap()
        d_qt = nc.dram_tensor("d_qt", [128, Q], F32, kind="ExternalOutput").ap()
        d_esm = nc.dram_tensor("d_esm", [128, Q], F32, kind="ExternalOutput").ap()
        d_psa = nc.dram_tensor("d_psa", [2 * 65, Q], F32, kind="ExternalOutput").ap()
        d_acc = nc.dram_tensor("d_acc", [128, Q], F32, kind="ExternalOutput").ap()
        d_v = nc.dram_tensor("d_v", [128, NH * 65], F32, kind="ExternalOutput").ap()

    with tile.TileContext(nc) as tc, ExitStack() as top:
        res = top.enter_context(tc.tile_pool(name="res", bufs=1))

        # ---- resident tiles -------------------------------------------------
        hsT_sb = []
        for hc in range(HC):
            t = res.tile([128, S], F32R, tag=f"hsT{hc}")
            nc.sync.dma_start(t[:], hsT[hc * 128:(hc + 1) * 128, :])
            hsT_sb.append(t)
        cos_sb = res.tile([128, S], F32, tag="cos")
        nc.sync.dma_start(cos_sb[:], cosk[:])
        sin_sb = res.tile([128, S], F32, tag="sin")
        nc.sync.dma_start(sin_sb[:], sink[:])
        em_sb = []
        for sc in range(SC16):
            t = res.tile([128, Q], F16, tag=f"em{sc}")
            nc.sync.dma_start(t[:], emask[sc * 128:(sc + 1) * 128, :])
            em_sb.append(t)
        # V_aug: per key-chunk, 16 heads x (64 cols + ones col)
        v_sb = [res.tile([128, NH * 65], F16, tag=f"v{sc}", name=f"v{sc}")
                for sc in range(SC16)]
        ones16 = res.tile([128, NH], F16, tag="ones16")
        nc.gpsimd.memset(ones16[:], 1.0)
        # per-pair attention output accumulator [hd(128), q]
        acc_sb = [res.tile([128, Q], F16, tag=f"acc{p}", name=f"acc{p}")
                  for p in range(PAIRS)]

        # ---- V projection ---------------------------------------------------
        with tc.tile_pool(name="wvp", bufs=1) as wvp, \
             tc.tile_pool(name="psv", bufs=4, space="PSUM") as psv:
            for g in range(4):  # groups of 4 heads = 256 cols
                wv_g = []
                for hc in range(HC):
                    t = wvp.tile([128, 256], F32R, tag=f"wv{hc}")
                    nc.sync.dma_start(
                        t[:], wv[hc * 128:(hc + 1) * 128, g * 256:(g + 1) * 256])
                    wv_g.append(t)
                for sc in range(SC16):
                    ps = psv.tile([128, 256], F32, tag="psv")
                    for hc in range(HC):
                        nc.tensor.matmul(
                            ps[:], hsT_sb[hc][:, sc * 128:(sc + 1) * 128],
                            wv_g[hc][:], start=(hc == 0), stop=(hc == HC - 1))
                    for hh in range(4):
                        h = 4 * g + hh
                        nc.scalar.copy(v_sb[sc][:, h * 65:h * 65 + 64],
                                       ps[:, hh * 64:(hh + 1) * 64])
                # ones columns for these heads
                for sc in range(SC16):
                    v3 = v_sb[sc][:].rearrange("p (h c) -> p h c", h=NH)
                    nc.gpsimd.tensor_copy(v3[:, 4 * g:4 * g + 4, 64],
                                          ones16[:, 4 * g:4 * g + 4])

        # ---- head-pair loop -------------------------------------------------
        with tc.tile_pool(name="wqk", bufs=2) as wqk, \
             tc.tile_pool(name="kt", bufs=2) as ktp, \
             tc.tile_pool(name="qt", bufs=2) as qtp, \
             tc.tile_pool(name="rope", bufs=2) as rope, \
             tc.tile_pool(name="expp", bufs=3) as expp, \
             tc.tile_pool(name="nrm", bufs=2) as nrm, \
             tc.tile_pool(name="psk", bufs=2, space="PSUM") as psk, \
             tc.tile_pool(name="pss", bufs=2, space="PSUM") as pss, \
             tc.tile_pool(name="psa", bufs=2, space="PSUM") as psa:

            def rope_apply(dst, ps, s0, n):
                """dst[:, s0:s0+n] = rope(ps) for a head pair [128, n]."""
                t1 = rope.tile([128, 512], F32, tag="t1")
                nc.vector.tensor_tensor(
                    t1[:, :n], ps[:, :n], cos_sb[:, s0:s0 + n], AluOpType.mult)
                t2 = rope.tile([128, 512], F32, tag="t2")
                for hb in (0, 64):
                    nc.vector.tensor_tensor(
                        t2[hb:hb + 32, :n], ps[hb + 32:hb + 64, :n],
                        sin_sb[hb:hb + 32, s0:s0 + n], AluOpType.mult)
                    nc.vector.tensor_tensor(
                        t2[hb + 32:hb + 64, :n], ps[hb:hb + 32, :n],
                        sin_sb[hb + 32:hb + 64, s0:s0 + n], AluOpType.mult)
                nc.vector.tensor_tensor(
                    dst[:, s0:s0 + n], t1[:, :n], t2[:, :n], AluOpType.add)

            for p in range(PAIRS):
                c0 = p * 128
                wk_p, wq_p = [], []
                for hc in range(HC):
                    t = wqk.tile([128, 128], F32R, tag=f"wk{hc}")
                    nc.sync.dma_start(t[:], wk[hc * 128:(hc + 1) * 128,
                                               c0:c0 + 128])
                    wk_p.append(t)
                    t = wqk.tile([128, 128], F32R, tag=f"wq{hc}")
                    nc.sync.dma_start(t[:], wq[hc * 128:(hc + 1) * 128,
                                               c0:c0 + 128])
                    wq_p.append(t)

                # K projection + RoPE -> kT pair-packed [128, S]
                kt_pr = ktp.tile([128, S], F32R, tag="kt")
                for sc in range(SC4):
                    ps = psk.tile([128, 512], F32, tag="psk")
                    for hc in range(HC):
                        nc.tensor.matmul(
                            ps[:], wk_p[hc][:],
                            hsT_sb[hc][:, sc * 512:(sc + 1) * 512],
                            start=(hc == 0), stop=(hc == HC - 1))
                    rope_apply(kt_pr, ps, sc * 512, 512)

                # Q projection + RoPE -> qT pair-packed [128, Q]
                qt_pr = qtp.tile([128, Q], F32R, tag="qt")
                ps = psk.tile([128, 512], F32, tag="psk")
                for hc in range(HC):
                    nc.tensor.matmul(ps[:], wq_p[hc][:], hsT_sb[hc][:, 0:Q],
                                     start=(hc == 0), stop=(hc == HC - 1))
                rope_apply(qt_pr, ps, 0, Q)

                if dbg and p == 0:
                    nc.sync.dma_start(d_kt[:], kt_pr[:].bitcast(F32))
                    nc.sync.dma_start(d_qt[:], qt_pr[:].bitcast(F32))

                # attention per head
                for half in range(2):
                    h = 2 * p + half
                    hb = half * 64
                    ps_a = psa.tile([65, Q], F32, tag="psa")
                    for sc in range(SC16):
                        ps_s = pss.tile([128, Q], F32, tag="pss")
                        nc.tensor.matmul(
                            ps_s[:], kt_pr[hb:hb + 64, sc * 128:(sc + 1) * 128],
                            qt_pr[hb:hb + 64, :], start=True, stop=True)
                        t_exp = expp.tile([128, Q], F16, tag="texp")
                        nc.scalar.activation(t_exp[:], ps_s[:], AF.Exp)
                        t_em = expp.tile([128, Q], F16, tag="tem")
                        nc.vector.tensor_tensor(t_em[:], t_exp[:], em_sb[sc][:],
                                                AluOpType.mult)
                        if dbg and p == 0 and half == 0 and sc == 0:
                            t_d = expp.tile([128, Q], F32, tag="tdbg")
                            nc.vector.tensor_copy(t_d[:], t_em[:])
                            nc.sync.dma_start(d_esm[:], t_d[:])
                        nc.tensor.matmul(ps_a[:], v_sb[sc][:, h * 65:h * 65 + 65],
                                         t_em[:], start=(sc == 0),
                                         stop=(sc == SC16 - 1))
                    # normalize by denominator row (64) and store to acc
                    if dbg and p == 0:
                        t_d2 = nrm.tile([65, Q], F32, tag="tdbg2")
                        nc.vector.tensor_copy(t_d2[:], ps_a[:])
                        nc.sync.dma_start(d_psa[half * 65:(half + 1) * 65, :],
                                          t_d2[:])
                    rec = nrm.tile([1, Q], F32, tag="rec")
                    nc.vector.reciprocal(rec[:], ps_a[64:65, :])
                    bc = nrm.tile([128, Q], F32, tag="bc")
                    nc.gpsimd.partition_broadcast(bc[:], rec[:])
                    nc.vector.tensor_tensor(acc_sb[p][hb:hb + 64, :],
                                            ps_a[0:64, :], bc[hb:hb + 64, :],
                                            AluOpType.mult)

        if dbg:
            with tc.tile_pool(name="dbgp", bufs=1) as dbgp:
                t_d3 = dbgp.tile([128, Q], F32, tag="td3")
                nc.vector.tensor_copy(t_d3[:], acc_sb[0][:])
                nc.sync.dma_start(d_acc[:], t_d3[:])
                t_d4 = dbgp.tile([128, NH * 65], F32, tag="td4")
                nc.vector.tensor_copy(t_d4[:], v_sb[0][:])
                nc.sync.dma_start(d_v[:], t_d4[:])

        # ---- output projection ---------------------------------------------
        with tc.tile_pool(name="wop", bufs=1) as wop, \
             tc.tile_pool(name="outp", bufs=3) as outp, \
             tc.tile_pool(name="pso", bufs=2, space="PSUM") as pso:
            wo_p = []
            for p in range(PAIRS):
                t = wop.tile([128, HID], F16, tag=f"wo{p}")
                nc.sync.dma_start(t[:], wo[p * 128:(p + 1) * 128, :])
                wo_p.append(t)
            for qc in range(Q // 128):
                for nn in range(2):
                    ps = pso.tile([128, 512], F32, tag="pso")
                    for p in range(PAIRS):
                        nc.tensor.matmul(
                            ps[:], acc_sb[p][:, qc * 128:(qc + 1) * 128],
                            wo_p[p][:, nn * 512:(nn + 1) * 512],
                            start=(p == 0), stop=(p == PAIRS - 1))
                    t_out = outp.tile([128, 512], F32, tag="tout")
                    nc.scalar.copy(t_out[:], ps[:])
                    nc.sync.dma_start(
                        out[qc * 128:(qc + 1) * 128, nn * 512:(nn + 1) * 512],
                        t_out[:])

    nc.compile()
    return nc


_NC_CACHE = None


def _get_program():
    global _NC_CACHE
    if _NC_CACHE is None:
        _NC_CACHE = build_program()
    return _NC_CACHE


def make_in_maps(hidden_states, attention_mask, position_ids, cos, sin,
                 Wq, Wk, Wv, Wo):
    hidden_states = np.asarray(hidden_states, np.float32)
    attention_mask = np.asarray(attention_mask, np.float32)
    position_ids = np.asarray(position_ids)
    cos = np.asarray(cos, np.float32)
    sin = np.asarray(sin, np.float32)
    wq_s = (np.asarray(Wq, np.float32) * SCALE).astype(np.float32)
    wk_ = np.ascontiguousarray(np.asarray(Wk, np.float32))
    wv_ = np.ascontiguousarray(np.asarray(Wv, np.float32))
    wo_ = np.ascontiguousarray(np.asarray(Wo, np.float32)).astype(np.float16)

    in_maps = []
    for b in range(B):
        hsT_b = hidden_states[b].T  # [HID, S]
        cos_b = cos[position_ids[b]]  # [S, HD]
        sin_b = sin[position_ids[b]]
        cosT = np.tile(cos_b.T, (2, 1))  # [128, S] (two heads stacked)
        # sign-folded sin for rotate_half: rows 0:32 get -sin, 32:64 get +sin
        sinT = sin_b.T.copy()
        sinT[0:32] *= -1.0
        sinT = np.tile(sinT, (2, 1))  # [128, S]
        maskT_b = attention_mask[b, 0].T  # [S(keys), S(queries)]
        for qc in range(4):
            q0 = qc * Q
            roll = -q0
            in_maps.append({
                "hsT": np.ascontiguousarray(np.roll(hsT_b, roll, axis=1)),
                "cosk": np.ascontiguousarray(np.roll(cosT, roll, axis=1)),
                "sink": np.ascontiguousarray(np.roll(sinT, roll, axis=1)),
                "emask": np.exp(
                    np.roll(maskT_b[:, q0:q0 + Q], roll, axis=0)
                ).astype(np.float16),
                "wq": wq_s, "wk": wk_, "wv": wv_, "wo": wo_,
            })
    return in_maps


def run(inputs: dict, trace: bool = False):
    nc = _get_program()
    in_maps = make_in_maps(**inputs)
    res = run_bass_kernel_spmd(nc, in_maps, list(range(N_CORES)), trace=trace)
    out = np.empty((B, S, HID), np.float32)
    for c in range(N_CORES):
        b, qc = c // 4, c % 4
        out[b, qc * Q:(qc + 1) * Q, :] = res.results[c]["out"]
    return out, res


def kernel(**inputs) -> np.ndarray:
    out, _ = run(inputs, trace=False)
    return out


# revision 7
# speedup vs baseline: 1.1511x; 1.1511x over previous
"""Multi-head attention (RoPE + causal-mask softmax) on 8 TRN2 NeuronCores.

Sharding: batch x q-chunk (2 batches x 4 chunks of 512 query rows). Each core
computes all 16 heads for its 512 query rows. K/V are recomputed per core
(whole batch), so there are no collectives; outputs are disjoint slices.

To make the program SPMD-uniform, each core's sequence axis is rolled so its
query window sits at s=0 (attention is permutation-invariant over the key
axis when K, V, mask and the RoPE tables are rolled together).
"""

from contextlib import ExitStack

import numpy as np

import concourse.bass as bass
import concourse.tile as tile
from concourse import bacc, mybir
from concourse.alu_op_type import AluOpType
from concourse.bass_utils import run_bass_kernel_spmd

AF = mybir.ActivationFunctionType
F32 = mybir.dt.float32
F32R = mybir.dt.float32r
F16 = mybir.dt.float16

B, S, HID, NH, HD = 2, 2048, 1024, 16, 64
SCALE = 1.0 / np.sqrt(HD)
N_CORES = 8
Q = 512          # query rows per core
HC = HID // 128  # hidden chunks (8)
PAIRS = NH // 2  # head pairs (8)
SC16 = S // 128  # key chunks of 128 (16)
SC4 = S // 512   # key chunks of 512 (4)


def build_program(dbg: bool = False):
    nc = bacc.Bacc("TRN2", target_bir_lowering=False, debug=False,
                   num_devices=N_CORES)

    hsT = nc.dram_tensor("hsT", [HID, S], F32R, kind="ExternalInput").ap()
    cosk = nc.dram_tensor("cosk", [128, S], F32, kind="ExternalInput").ap()
    sink = nc.dram_tensor("sink", [128, S], F32, kind="ExternalInput").ap()
    emask = nc.dram_tensor("emask", [S, Q], F16, kind="ExternalInput").ap()
    wq = nc.dram_tensor("wq", [HID, HID], F32R, kind="ExternalInput").ap()
    wk = nc.dram_tensor("wk", [HID, HID], F32R, kind="ExternalInput").ap()
    wv = nc.dram_tensor("wv", [HID, HID], F32R, kind="ExternalInput").ap()
    wo = nc.dram_tensor("wo", [HID, HID], F16, kind="ExternalInput").ap()
    out = nc.dram_tensor("out", [Q, HID], F32, kind="ExternalOutput").ap()
    if dbg:
        d_kt = nc.dram_tensor("d_kt", [128, S], F32, kind="ExternalOutput").ap()
        d_qt = nc.dram_tensor("d_qt", [128, Q], F32, kind="ExternalOutput").ap()
        d_esm = nc.dram_tensor("d_esm", [128, Q], F32, kind="ExternalOutput").ap()
        d_psa = nc.dram_tensor("d_psa", [2 * 65, Q], F32, kind="ExternalOutput").ap()
        d_acc = nc.dram_tensor("d_acc", [128, Q], F32, kind="ExternalOutput").ap()
        d_v = nc.dram_tensor("d_v", [128, NH * 65], F32, kind="ExternalOutput").ap()

    with tile.TileContext(nc) as tc, ExitStack() as top:
        res = top.enter_context(tc.tile_pool(name="res", bufs=1))

        # ---- resident tiles -------------------------------------------------
        hsT_sb = []
        for hc in range(HC):
            t = res.tile([128, S], F32R, tag=f"hsT{hc}")
            nc.sync.dma_start(t[:], hsT[hc * 128:(hc + 1) * 128, :])
            hsT_sb.append(t)
        cos_sb = res.tile([128, S], F32, tag="cos")
        nc.sync.dma_start(cos_sb[:], cosk[:])
        sin_sb = res.tile([128, S], F32, tag="sin")
        nc.sync.dma_start(sin_sb[:], sink[:])
        em_sb = []
        for sc in range(SC16):
            t = res.tile([128, Q], F16, tag=f"em{sc}")
            nc.sync.dma_start(t[:], emask[sc * 128:(sc + 1) * 128, :])
            em_sb.append(t)
        # V_aug: per key-chunk, 16 heads x (64 cols + ones col)
        v_sb = [res.tile([128, NH * 65], F16, tag=f"v{sc}", name=f"v{sc}")
                for sc in range(SC16)]
        ones16 = res.tile([128, NH], F16, tag="ones16")
        nc.gpsimd.memset(ones16[:], 1.0)
        # per-pair attention output accumulator [hd(128), q]
        acc_sb = [res.tile([128, Q], F16, tag=f"acc{p}", name=f"acc{p}")
                  for p in range(PAIRS)]

        # ---- V projection ---------------------------------------------------
        with tc.tile_pool(name="wvp", bufs=1) as wvp, \
             tc.tile_pool(name="psv", bufs=4, space="PSUM") as psv:
            for g in range(4):  # groups of 4 heads = 256 cols
                wv_g = []
                for hc in range(HC):
                    t = wvp.tile([128, 256], F32R, tag=f"wv{hc}")
                    nc.sync.dma_start(
                        t[:], wv[hc * 128:(hc + 1) * 128, g * 256:(g + 1) * 256])
                    wv_g.append(t)
                for sc in range(SC16):
                    ps = psv.tile([128, 256], F32, tag="psv")
                    for hc in range(HC):
                        nc.tensor.matmul(
                            ps[:], hsT_sb[hc][:, sc * 128:(sc + 1) * 128],
                            wv_g[hc][:], start=(hc == 0), stop=(hc == HC - 1))
                    for hh in range(4):
                        h = 4 * g + hh
                        nc.vector.tensor_copy(v_sb[sc][:, h * 65:h * 65 + 64],
                                              ps[:, hh * 64:(hh + 1) * 64])
                # ones columns for these heads
                for sc in range(SC16):
                    v3 = v_sb[sc][:].rearrange("p (h c) -> p h c", h=NH)
                    nc.gpsimd.tensor_copy(v3[:, 4 * g:4 * g + 4, 64],
                                          ones16[:, 4 * g:4 * g + 4])

        # ---- head-pair loop -------------------------------------------------
        with tc.tile_pool(name="wqk", bufs=2) as wqk, \
             tc.tile_pool(name="kt", bufs=2) as ktp, \
             tc.tile_pool(name="qt", bufs=2) as qtp, \
             tc.tile_pool(name="rope", bufs=2) as rope, \
             tc.tile_pool(name="expp", bufs=6) as expp, \
             tc.tile_pool(name="nrm", bufs=2) as nrm, \
             tc.tile_pool(name="psk", bufs=2, space="PSUM") as psk, \
             tc.tile_pool(name="pss", bufs=4, space="PSUM") as pss, \
             tc.tile_pool(name="psa", bufs=1, space="PSUM") as psa:

            def rope_apply(dst, ps, s0, n):
                """dst[:, s0:s0+n] = rope(ps) for a head pair [128, n]."""
                t1 = rope.tile([128, 512], F32, tag="t1")
                nc.vector.tensor_tensor(
                    t1[:, :n], ps[:, :n], cos_sb[:, s0:s0 + n], AluOpType.mult)
                t2 = rope.tile([128, 512], F32, tag="t2")
                for hb in (0, 64):
                    nc.vector.tensor_tensor(
                        t2[hb:hb + 32, :n], ps[hb + 32:hb + 64, :n],
                        sin_sb[hb:hb + 32, s0:s0 + n], AluOpType.mult)
                    nc.vector.tensor_tensor(
                        t2[hb + 32:hb + 64, :n], ps[hb:hb + 32, :n],
                        sin_sb[hb + 32:hb + 64, s0:s0 + n], AluOpType.mult)
                nc.vector.tensor_tensor(
                    dst[:, s0:s0 + n], t1[:, :n], t2[:, :n], AluOpType.add)

            for p in range(PAIRS):
                c0 = p * 128
                wk_p, wq_p = [], []
                for hc in range(HC):
                    t = wqk.tile([128, 128], F32R, tag=f"wk{hc}")
                    nc.sync.dma_start(t[:], wk[hc * 128:(hc + 1) * 128,
                                               c0:c0 + 128])
                    wk_p.append(t)
                    t = wqk.tile([128, 128], F32R, tag=f"wq{hc}")
                    nc.sync.dma_start(t[:], wq[hc * 128:(hc + 1) * 128,
                                               c0:c0 + 128])
                    wq_p.append(t)

                # K projection + RoPE -> kT pair-packed [128, S]
                kt_pr = ktp.tile([128, S], F32R, tag="kt")
                for sc in range(SC4):
                    ps = psk.tile([128, 512], F32, tag="psk")
                    for hc in range(HC):
                        nc.tensor.matmul(
                            ps[:], wk_p[hc][:],
                            hsT_sb[hc][:, sc * 512:(sc + 1) * 512],
                            start=(hc == 0), stop=(hc == HC - 1))
                    rope_apply(kt_pr, ps, sc * 512, 512)

                # Q projection + RoPE -> qT pair-packed [128, Q]
                qt_pr = qtp.tile([128, Q], F32R, tag="qt")
                ps = psk.tile([128, 512], F32, tag="psk")
                for hc in range(HC):
                    nc.tensor.matmul(ps[:], wq_p[hc][:], hsT_sb[hc][:, 0:Q],
                                     start=(hc == 0), stop=(hc == HC - 1))
                rope_apply(qt_pr, ps, 0, Q)

                if dbg and p == 0:
                    nc.sync.dma_start(d_kt[:], kt_pr[:].bitcast(F32))
                    nc.sync.dma_start(d_qt[:], qt_pr[:].bitcast(F32))

                # attention per head
                ps_a2 = [psa.tile([65, Q], F32, tag=f"psa{half}",
                                  name=f"psa{half}") for half in range(2)]
                for sc in range(SC16):
                    for half in range(2):
                        h = 2 * p + half
                        hb = half * 64
                        ps_s = pss.tile([128, Q], F32, tag="pss")
                        nc.tensor.matmul(
                            ps_s[:], kt_pr[hb:hb + 64, sc * 128:(sc + 1) * 128],
                            qt_pr[hb:hb + 64, :], start=True, stop=True)
                        t_exp = expp.tile([128, Q], F16, tag="texp")
                        nc.scalar.activation(t_exp[:], ps_s[:], AF.Exp)
                        t_em = expp.tile([128, Q], F16, tag="tem")
                        nc.vector.tensor_tensor(t_em[:], t_exp[:], em_sb[sc][:],
                                                AluOpType.mult)
                        if dbg and p == 0 and half == 0 and sc == 0:
                            t_d = expp.tile([128, Q], F32, tag="tdbg")
                            nc.vector.tensor_copy(t_d[:], t_em[:])
                            nc.sync.dma_start(d_esm[:], t_d[:])
                        nc.tensor.matmul(ps_a2[half][:],
                                         v_sb[sc][:, h * 65:h * 65 + 65],
                                         t_em[:], start=(sc == 0),
                                         stop=(sc == SC16 - 1))
                for half in range(2):
                    hb = half * 64
                    ps_a = ps_a2[half]
                    # normalize by denominator row (64) and store to acc
                    if dbg and p == 0:
                        t_d2 = nrm.tile([65, Q], F32, tag="tdbg2")
                        nc.vector.tensor_copy(t_d2[:], ps_a[:])
                        nc.sync.dma_start(d_psa[half * 65:(half + 1) * 65, :],
                                          t_d2[:])
                    den = nrm.tile([1, Q], F32, tag="den")
                    nc.scalar.copy(den[:], ps_a[64:65, :])
                    bc = nrm.tile([128, Q], F32, tag="bc")
                    nc.gpsimd.partition_broadcast(bc[:], den[:])
                    rec = nrm.tile([128, Q], F32, tag="recb")
                    nc.vector.reciprocal(rec[hb:hb + 64, :], bc[hb:hb + 64, :])
                    nc.vector.tensor_tensor(acc_sb[p][hb:hb + 64, :],
                                            ps_a[0:64, :], rec[hb:hb + 64, :],
                                            AluOpType.mult)

        if dbg:
            with tc.tile_pool(name="dbgp", bufs=1) as dbgp:
                t_d3 = dbgp.tile([128, Q], F32, tag="td3")
                nc.vector.tensor_copy(t_d3[:], acc_sb[0][:])
                nc.sync.dma_start(d_acc[:], t_d3[:])
                t_d4 = dbgp.tile([128, NH * 65], F32, tag="td4")
                nc.vector.tensor_copy(t_d4[:], v_sb[0][:])
                nc.sync.dma_start(d_v[:], t_d4[:])

        # ---- output projection ---------------------------------------------
        with tc.tile_pool(name="wop", bufs=1) as wop, \
             tc.tile_pool(name="outp", bufs=3) as outp, \
             tc.tile_pool(name="pso", bufs=2, space="PSUM") as pso:
            wo_p = []
            for p in range(PAIRS):
                t = wop.tile([128, HID], F16, tag=f"wo{p}")
                nc.sync.dma_start(t[:], wo[p * 128:(p + 1) * 128, :])
                wo_p.append(t)
            for qc in range(Q // 128):
                for nn in range(2):
                    ps = pso.tile([128, 512], F32, tag="pso")
                    for p in range(PAIRS):
                        nc.tensor.matmul(
                            ps[:], acc_sb[p][:, qc * 128:(qc + 1) * 128],
                            wo_p[p][:, nn * 512:(nn + 1) * 512],
                            start=(p == 0), stop=(p == PAIRS - 1))
                    t_out = outp.tile([128, 512], F32, tag="tout")
                    nc.vector.tensor_copy(t_out[:], ps[:])
                    nc.sync.dma_start(
                        out[qc * 128:(qc + 1) * 128, nn * 512:(nn + 1) * 512],
                        t_out[:])

    nc.compile()
    return nc


_NC_CACHE = None


def _get_program():
    global _NC_CACHE
    if _NC_CACHE is None:
        _NC_CACHE = build_program()
    return _NC_CACHE


def make_in_maps(hidden_states, attention_mask, position_ids, cos, sin,
                 Wq, Wk, Wv, Wo):
    hidden_states = np.asarray(hidden_states, np.float32)
    attention_mask = np.asarray(attention_mask, np.float32)
    position_ids = np.asarray(position_ids)
    cos = np.asarray(cos, np.float32)
    sin = np.asarray(sin, np.float32)
    wq_s = (np.asarray(Wq, np.float32) * SCALE).astype(np.float32)
    wk_ = np.ascontiguousarray(np.asarray(Wk, np.float32))
    wv_ = np.ascontiguousarray(np.asarray(Wv, np.float32))
    wo_ = np.ascontiguousarray(np.asarray(Wo, np.float32)).astype(np.float16)

    in_maps = []
    for b in range(B):
        hsT_b = hidden_states[b].T  # [HID, S]
        cos_b = cos[position_ids[b]]  # [S, HD]
        sin_b = sin[position_ids[b]]
        cosT = np.tile(cos_b.T, (2, 1))  # [128, S] (two heads stacked)
        # sign-folded sin for rotate_half: rows 0:32 get -sin, 32:64 get +sin
        sinT = sin_b.T.copy()
        sinT[0:32] *= -1.0
        sinT = np.tile(sinT, (2, 1))  # [128, S]
        maskT_b = attention_mask[b, 0].T  # [S(keys), S(queries)]
        for qc in range(4):
            q0 = qc * Q
            roll = -q0
            in_maps.append({
                "hsT": np.ascontiguousarray(np.roll(hsT_b, roll, axis=1)),
                "cosk": np.ascontiguousarray(np.roll(cosT, roll, axis=1)),
                "sink": np.ascontiguousarray(np.roll(sinT, roll, axis=1)),
                "emask": np.exp(
                    np.roll(maskT_b[:, q0:q0 + Q], roll, axis=0)
                ).astype(np.float16),
                "wq": wq_s, "wk": wk_, "wv": wv_, "wo": wo_,
            })
    return in_maps


def run(inputs: dict, trace: bool = False):
    nc = _get_program()
    in_maps = make_in_maps(**inputs)
    res = run_bass_kernel_spmd(nc, in_maps, list(range(N_CORES)), trace=trace)
    out = np.empty((B, S, HID), np.float32)
    for c in range(N_CORES):
        b, qc = c // 4, c % 4
        out[b, qc * Q:(qc + 1) * Q, :] = res.results[c]["out"]
    return out, res


def kernel(**inputs) -> np.ndarray:
    out, _ = run(inputs, trace=False)
    return out


# revision 10
# speedup vs baseline: 1.1805x; 1.0256x over previous
"""Multi-head attention (RoPE + causal-mask softmax) on 8 TRN2 NeuronCores.

Sharding: batch x q-chunk (2 batches x 4 chunks of 512 query rows). Each core
computes all 16 heads for its 512 query rows. K/V are recomputed per core
(whole batch), so there are no collectives; outputs are disjoint slices.

To make the program SPMD-uniform, each core's sequence axis is rolled so its
query window sits at s=0 (attention is permutation-invariant over the key
axis when K, V, mask and the RoPE tables are rolled together).
"""

from contextlib import ExitStack

import numpy as np

import concourse.bass as bass
import concourse.tile as tile
from concourse import bacc, mybir
from concourse.alu_op_type import AluOpType
from concourse.bass_utils import run_bass_kernel_spmd

AF = mybir.ActivationFunctionType
F32 = mybir.dt.float32
F32R = mybir.dt.float32r
F16 = mybir.dt.float16
BF16 = mybir.dt.bfloat16

B, S, HID, NH, HD = 2, 2048, 1024, 16, 64
SCALE = 1.0 / np.sqrt(HD)
N_CORES = 8
Q = 512          # query rows per core
HC = HID // 128  # hidden chunks (8)
PAIRS = NH // 2  # head pairs (8)
SC16 = S // 128  # key chunks of 128 (16)
SC4 = S // 512   # key chunks of 512 (4)


def build_program(dbg: bool = False):
    nc = bacc.Bacc("TRN2", target_bir_lowering=False, debug=False,
                   num_devices=N_CORES)

    hsT = nc.dram_tensor("hsT", [HID, S], BF16, kind="ExternalInput").ap()
    cosk = nc.dram_tensor("cosk", [128, S], F32, kind="ExternalInput").ap()
    sink = nc.dram_tensor("sink", [128, S], F32, kind="ExternalInput").ap()
    emask = nc.dram_tensor("emask", [S, Q], F16, kind="ExternalInput").ap()
    wq = nc.dram_tensor("wq", [HID, HID], BF16, kind="ExternalInput").ap()
    wk = nc.dram_tensor("wk", [HID, HID], BF16, kind="ExternalInput").ap()
    wv = nc.dram_tensor("wv", [HID, HID], BF16, kind="ExternalInput").ap()
    wo = nc.dram_tensor("wo", [HID, HID], F16, kind="ExternalInput").ap()
    out = nc.dram_tensor("out", [Q, HID], F32, kind="ExternalOutput").ap()
    if dbg:
        d_kt = nc.dram_tensor("d_kt", [128, S], F32, kind="ExternalOutput").ap()
        d_qt = nc.dram_tensor("d_qt", [128, Q], F32, kind="ExternalOutput").ap()
        d_esm = nc.dram_tensor("d_esm", [128, Q], F32, kind="ExternalOutput").ap()
        d_psa = nc.dram_tensor("d_psa", [2 * 65, Q], F32, kind="ExternalOutput").ap()
        d_acc = nc.dram_tensor("d_acc", [128, Q], F32, kind="ExternalOutput").ap()
        d_v = nc.dram_tensor("d_v", [128, NH * 65], F32, kind="ExternalOutput").ap()

    with tile.TileContext(nc) as tc, ExitStack() as top:
        res = top.enter_context(tc.tile_pool(name="res", bufs=1))

        # ---- resident tiles -------------------------------------------------
        hsT_sb = []
        for hc in range(HC):
            t = res.tile([128, S], BF16, tag=f"hsT{hc}")
            nc.sync.dma_start(t[:], hsT[hc * 128:(hc + 1) * 128, :])
            hsT_sb.append(t)
        cos_sb = res.tile([128, S], F32, tag="cos")
        nc.sync.dma_start(cos_sb[:], cosk[:])
        sin_sb = res.tile([128, S], F32, tag="sin")
        nc.sync.dma_start(sin_sb[:], sink[:])
        em_sb = []
        for sc in range(SC16):
            t = res.tile([128, Q], F16, tag=f"em{sc}")
            nc.sync.dma_start(t[:], emask[sc * 128:(sc + 1) * 128, :])
            em_sb.append(t)
        # V_aug: per key-chunk, 16 heads x (64 cols + ones col)
        v_sb = [res.tile([128, NH * 65], F16, tag=f"v{sc}", name=f"v{sc}")
                for sc in range(SC16)]
        ones16 = res.tile([128, NH], F16, tag="ones16")
        nc.gpsimd.memset(ones16[:], 1.0)
        # per-pair attention output accumulator [hd(128), q]
        acc_sb = [res.tile([128, Q], F16, tag=f"acc{p}", name=f"acc{p}")
                  for p in range(PAIRS)]

        # ---- V projection ---------------------------------------------------
        with tc.tile_pool(name="wvp", bufs=1) as wvp, \
             tc.tile_pool(name="psv", bufs=4, space="PSUM") as psv:
            for g in range(2):  # groups of 8 heads = 512 cols
                wv_g = []
                for hc in range(HC):
                    t = wvp.tile([128, 512], BF16, tag=f"wv{hc}")
                    nc.sync.dma_start(
                        t[:], wv[hc * 128:(hc + 1) * 128, g * 512:(g + 1) * 512])
                    wv_g.append(t)
                for sc in range(SC16):
                    ps = psv.tile([128, 512], F32, tag="psv")
                    for hc in range(HC):
                        nc.tensor.matmul(
                            ps[:], hsT_sb[hc][:, sc * 128:(sc + 1) * 128],
                            wv_g[hc][:], start=(hc == 0), stop=(hc == HC - 1))
                    for hh in range(8):
                        h = 8 * g + hh
                        nc.vector.tensor_copy(v_sb[sc][:, h * 65:h * 65 + 64],
                                              ps[:, hh * 64:(hh + 1) * 64])
                # ones columns for these heads
                for sc in range(SC16):
                    v3 = v_sb[sc][:].rearrange("p (h c) -> p h c", h=NH)
                    nc.gpsimd.tensor_copy(v3[:, 8 * g:8 * g + 8, 64],
                                          ones16[:, 8 * g:8 * g + 8])

        # ---- head-pair loop -------------------------------------------------
        with tc.tile_pool(name="wqk", bufs=2) as wqk, \
             tc.tile_pool(name="kt", bufs=2) as ktp, \
             tc.tile_pool(name="qt", bufs=2) as qtp, \
             tc.tile_pool(name="rope", bufs=2) as rope, \
             tc.tile_pool(name="expp", bufs=6) as expp, \
             tc.tile_pool(name="nrm", bufs=2) as nrm, \
             tc.tile_pool(name="psk", bufs=2, space="PSUM") as psk, \
             tc.tile_pool(name="pss", bufs=4, space="PSUM") as pss, \
             tc.tile_pool(name="psa", bufs=1, space="PSUM") as psa:

            def rope_apply(dst, ps, s0, n):
                """dst[:, s0:s0+n] = rope(ps) for a head pair [128, n]."""
                t1 = rope.tile([128, 512], F32, tag="t1")
                nc.vector.tensor_tensor(
                    t1[:, :n], ps[:, :n], cos_sb[:, s0:s0 + n], AluOpType.mult)
                t2 = rope.tile([128, 512], F32, tag="t2")
                for hb in (0, 64):
                    nc.vector.tensor_tensor(
                        t2[hb:hb + 32, :n], ps[hb + 32:hb + 64, :n],
                        sin_sb[hb:hb + 32, s0:s0 + n], AluOpType.mult)
                    nc.vector.tensor_tensor(
                        t2[hb + 32:hb + 64, :n], ps[hb:hb + 32, :n],
                        sin_sb[hb + 32:hb + 64, s0:s0 + n], AluOpType.mult)
                nc.vector.tensor_tensor(
                    dst[:, s0:s0 + n], t1[:, :n], t2[:, :n], AluOpType.add)

            for p in range(PAIRS):
                c0 = p * 128
                wk_p, wq_p = [], []
                for hc in range(HC):
                    t = wqk.tile([128, 128], BF16, tag=f"wk{hc}")
                    nc.sync.dma_start(t[:], wk[hc * 128:(hc + 1) * 128,
                                               c0:c0 + 128])
                    wk_p.append(t)
                    t = wqk.tile([128, 128], BF16, tag=f"wq{hc}")
                    nc.sync.dma_start(t[:], wq[hc * 128:(hc + 1) * 128,
                                               c0:c0 + 128])
                    wq_p.append(t)

                # K projection + RoPE -> kT pair-packed [128, S]
                kt_pr = ktp.tile([128, S], BF16, tag="kt")
                for sc in range(SC4):
                    ps = psk.tile([128, 512], F32, tag="psk")
                    for hc in range(HC):
                        nc.tensor.matmul(
                            ps[:], wk_p[hc][:],
                            hsT_sb[hc][:, sc * 512:(sc + 1) * 512],
                            start=(hc == 0), stop=(hc == HC - 1))
                    rope_apply(kt_pr, ps, sc * 512, 512)

                # Q projection + RoPE -> qT pair-packed [128, Q]
                qt_pr = qtp.tile([128, Q], BF16, tag="qt")
                ps = psk.tile([128, 512], F32, tag="psk")
                for hc in range(HC):
                    nc.tensor.matmul(ps[:], wq_p[hc][:], hsT_sb[hc][:, 0:Q],
                                     start=(hc == 0), stop=(hc == HC - 1))
                rope_apply(qt_pr, ps, 0, Q)

                if dbg and p == 0:
                    nc.sync.dma_start(d_kt[:], kt_pr[:].bitcast(F32))
                    nc.sync.dma_start(d_qt[:], qt_pr[:].bitcast(F32))

                # attention per head
                ps_a2 = [psa.tile([65, Q], F32, tag=f"psa{half}",
                                  name=f"psa{half}") for half in range(2)]
                for sc in range(SC16):
                    for half in range(2):
                        h = 2 * p + half
                        hb = half * 64
                        ps_s = pss.tile([128, Q], F32, tag="pss")
                        nc.tensor.matmul(
                            ps_s[:], kt_pr[hb:hb + 64, sc * 128:(sc + 1) * 128],
                            qt_pr[hb:hb + 64, :], start=True, stop=True)
                        t_exp = expp.tile([128, Q], F16, tag="texp")
                        nc.scalar.activation(t_exp[:], ps_s[:], AF.Exp)
                        t_em = expp.tile([128, Q], F16, tag="tem")
                        nc.vector.tensor_tensor(t_em[:], t_exp[:], em_sb[sc][:],
                                                AluOpType.mult)
                        if dbg and p == 0 and half == 0 and sc == 0:
                            t_d = expp.tile([128, Q], F32, tag="tdbg")
                            nc.vector.tensor_copy(t_d[:], t_em[:])
                            nc.sync.dma_start(d_esm[:], t_d[:])
                        nc.tensor.matmul(ps_a2[half][:],
                                         v_sb[sc][:, h * 65:h * 65 + 65],
                                         t_em[:], start=(sc == 0),
                                         stop=(sc == SC16 - 1))
                for half in range(2):
                    hb = half * 64
                    ps_a = ps_a2[half]
                    # normalize by denominator row (64) and store to acc
                    if dbg and p == 0:
                        t_d2 = nrm.tile([65, Q], F32, tag="tdbg2")
                        nc.vector.tensor_copy(t_d2[:], ps_a[:])
                        nc.sync.dma_start(d_psa[half * 65:(half + 1) * 65, :],
                                          t_d2[:])
                    den = nrm.tile([1, Q], F32, tag="den")
                    nc.scalar.copy(den[:], ps_a[64:65, :])
                    bc = nrm.tile([128, Q], F32, tag="bc")
                    nc.gpsimd.partition_broadcast(bc[:], den[:])
                    rec = nrm.tile([128, Q], F32, tag="recb")
                    nc.vector.reciprocal(rec[hb:hb + 64, :], bc[hb:hb + 64, :])
                    nc.vector.tensor_tensor(acc_sb[p][hb:hb + 64, :],
                                            ps_a[0:64, :], rec[hb:hb + 64, :],
                                            AluOpType.mult)

        if dbg:
            with tc.tile_pool(name="dbgp", bufs=1) as dbgp:
                t_d3 = dbgp.tile([128, Q], F32, tag="td3")
                nc.vector.tensor_copy(t_d3[:], acc_sb[0][:])
                nc.sync.dma_start(d_acc[:], t_d3[:])
                t_d4 = dbgp.tile([128, NH * 65], F32, tag="td4")
                nc.vector.tensor_copy(t_d4[:], v_sb[0][:])
                nc.sync.dma_start(d_v[:], t_d4[:])

        # ---- output projection ---------------------------------------------
        with tc.tile_pool(name="wop", bufs=1) as wop, \
             tc.tile_pool(name="outp", bufs=3) as outp, \
             tc.tile_pool(name="pso", bufs=2, space="PSUM") as pso:
            wo_p = []
            for p in range(PAIRS):
                t = wop.tile([128, HID], F16, tag=f"wo{p}")
                nc.sync.dma_start(t[:], wo[p * 128:(p + 1) * 128, :])
                wo_p.append(t)
            for qc in range(Q // 128):
                for nn in range(2):
                    ps = pso.tile([128, 512], F32, tag="pso")
                    for p in range(PAIRS):
                        nc.tensor.matmul(
                            ps[:], acc_sb[p][:, qc * 128:(qc + 1) * 128],
                            wo_p[p][:, nn * 512:(nn + 1) * 512],
                            start=(p == 0), stop=(p == PAIRS - 1))
                    t_out = outp.tile([128, 512], F32, tag="tout")
                    nc.vector.tensor_copy(t_out[:], ps[:])
                    nc.sync.dma_start(
                        out[qc * 128:(qc + 1) * 128, nn * 512:(nn + 1) * 512],
                        t_out[:])

    nc.compile()
    return nc


_NC_CACHE = None


def _get_program():
    global _NC_CACHE
    if _NC_CACHE is None:
        _NC_CACHE = build_program()
    return _NC_CACHE


def make_in_maps(hidden_states, attention_mask, position_ids, cos, sin,
                 Wq, Wk, Wv, Wo):
    import ml_dtypes
    bf16 = ml_dtypes.bfloat16
    hidden_states = np.asarray(hidden_states, np.float32)
    attention_mask = np.asarray(attention_mask, np.float32)
    position_ids = np.asarray(position_ids)
    cos = np.asarray(cos, np.float32)
    sin = np.asarray(sin, np.float32)
    wq_s = (np.asarray(Wq, np.float32) * SCALE).astype(bf16)
    wk_ = np.ascontiguousarray(np.asarray(Wk, np.float32)).astype(bf16)
    wv_ = np.ascontiguousarray(np.asarray(Wv, np.float32)).astype(bf16)
    wo_ = np.ascontiguousarray(np.asarray(Wo, np.float32)).astype(np.float16)

    in_maps = []
    for b in range(B):
        hsT_b = hidden_states[b].T  # [HID, S]
        cos_b = cos[position_ids[b]]  # [S, HD]
        sin_b = sin[position_ids[b]]
        cosT = np.tile(cos_b.T, (2, 1))  # [128, S] (two heads stacked)
        # sign-folded sin for rotate_half: rows 0:32 get -sin, 32:64 get +sin
        sinT = sin_b.T.copy()
        sinT[0:32] *= -1.0
        sinT = np.tile(sinT, (2, 1))  # [128, S]
        maskT_b = attention_mask[b, 0].T  # [S(keys), S(queries)]
        for qc in range(4):
            q0 = qc * Q
            roll = -q0
            in_maps.append({
                "hsT": np.ascontiguousarray(np.roll(hsT_b, roll, axis=1)).astype(bf16),
                "cosk": np.ascontiguousarray(np.roll(cosT, roll, axis=1)),
                "sink": np.ascontiguousarray(np.roll(sinT, roll, axis=1)),
                "emask": np.exp(
                    np.roll(maskT_b[:, q0:q0 + Q], roll, axis=0)
                ).astype(np.float16),
                "wq": wq_s, "wk": wk_, "wv": wv_, "wo": wo_,
            })
    return in_maps


def run(inputs: dict, trace: bool = False):
    nc = _get_program()
    in_maps = make_in_maps(**inputs)
    res = run_bass_kernel_spmd(nc, in_maps, list(range(N_CORES)), trace=trace)
    out = np.empty((B, S, HID), np.float32)
    for c in range(N_CORES):
        b, qc = c // 4, c % 4
        out[b, qc * Q:(qc + 1) * Q, :] = res.results[c]["out"]
    return out, res


def kernel(**inputs) -> np.ndarray:
    out, _ = run(inputs, trace=False)
    return out


# revision 15
# speedup vs baseline: 1.1855x; 1.0042x over previous
"""Multi-head attention (RoPE + causal-mask softmax) on 8 TRN2 NeuronCores.

Sharding: batch x q-chunk (2 batches x 4 chunks of 512 query rows). Each core
computes all 16 heads for its 512 query rows. K/V are recomputed per core
(whole batch), so there are no collectives; outputs are disjoint slices.

To make the program SPMD-uniform, each core's sequence axis is rolled so its
query window sits at s=0 (attention is permutation-invariant over the key
axis when K, V, mask and the RoPE tables are rolled together).
"""

from contextlib import ExitStack

import numpy as np

import concourse.bass as bass
import concourse.tile as tile
from concourse import bacc, mybir
from concourse.alu_op_type import AluOpType
from concourse.bass_utils import run_bass_kernel_spmd

AF = mybir.ActivationFunctionType
F32 = mybir.dt.float32
F32R = mybir.dt.float32r
F16 = mybir.dt.float16
BF16 = mybir.dt.bfloat16

B, S, HID, NH, HD = 2, 2048, 1024, 16, 64
SCALE = 1.0 / np.sqrt(HD)
N_CORES = 8
Q = 512          # query rows per core
HC = HID // 128  # hidden chunks (8)
PAIRS = NH // 2  # head pairs (8)
SC16 = S // 128  # key chunks of 128 (16)
SC4 = S // 512   # key chunks of 512 (4)


def build_program(dbg: bool = False):
    nc = bacc.Bacc("TRN2", target_bir_lowering=False, debug=False,
                   num_devices=N_CORES)

    hsT = nc.dram_tensor("hsT", [HID, S], BF16, kind="ExternalInput").ap()
    cosk = nc.dram_tensor("cosk", [128, S], F32, kind="ExternalInput").ap()
    sink = nc.dram_tensor("sink", [128, S], F32, kind="ExternalInput").ap()
    emask = nc.dram_tensor("emask", [S, Q], F16, kind="ExternalInput").ap()
    wq = nc.dram_tensor("wq", [HID, HID], BF16, kind="ExternalInput").ap()
    wk = nc.dram_tensor("wk", [HID, HID], BF16, kind="ExternalInput").ap()
    wv = nc.dram_tensor("wv", [HID, HID], BF16, kind="ExternalInput").ap()
    wo = nc.dram_tensor("wo", [HID, HID], F16, kind="ExternalInput").ap()
    sel = nc.dram_tensor("sel", [NH, HID], F32R, kind="ExternalInput").ap()
    out = nc.dram_tensor("out", [Q, HID], F32, kind="ExternalOutput").ap()
    if dbg:
        d_kt = nc.dram_tensor("d_kt", [128, S], F32, kind="ExternalOutput").ap()
        d_qt = nc.dram_tensor("d_qt", [128, Q], F32, kind="ExternalOutput").ap()
        d_esm = nc.dram_tensor("d_esm", [128, Q], F32, kind="ExternalOutput").ap()
        d_psa = nc.dram_tensor("d_psa", [2 * 65, Q], F32, kind="ExternalOutput").ap()
        d_acc = nc.dram_tensor("d_acc", [128, Q], F32, kind="ExternalOutput").ap()
        d_v = nc.dram_tensor("d_v", [128, NH * 65], F32, kind="ExternalOutput").ap()

    with tile.TileContext(nc) as tc, ExitStack() as top:
        res = top.enter_context(tc.tile_pool(name="res", bufs=1))

        # ---- resident tiles -------------------------------------------------
        hsT_sb = []
        for hc in range(HC):
            t = res.tile([128, S], BF16, tag=f"hsT{hc}")
            nc.sync.dma_start(t[:], hsT[hc * 128:(hc + 1) * 128, :])
            hsT_sb.append(t)
        cos_sb = res.tile([128, S], F32, tag="cos")
        nc.sync.dma_start(cos_sb[:], cosk[:])
        sin_sb = res.tile([128, S], F32, tag="sin")
        nc.sync.dma_start(sin_sb[:], sink[:])
        em_sb = []
        for sc in range(SC16):
            t = res.tile([128, Q], F16, tag=f"em{sc}")
            nc.sync.dma_start(t[:], emask[sc * 128:(sc + 1) * 128, :])
            em_sb.append(t)
        # V_aug: per key-chunk, 16 heads x (64 cols + ones col)
        v_sb = [res.tile([128, NH * 65], F16, tag=f"v{sc}", name=f"v{sc}")
                for sc in range(SC16)]
        ones16 = res.tile([128, NH], F16, tag="ones16")
        nc.gpsimd.memset(ones16[:], 1.0)
        # per-pair attention output accumulator [hd(128), q] (unnormalized)
        acc_sb = [res.tile([128, Q], F16, tag=f"acc{p}", name=f"acc{p}")
                  for p in range(PAIRS)]
        den_all = res.tile([NH, Q], F32, tag="den_all")
        wk_sb, wq_sb = [], []
        for hc in range(HC):
            t = res.tile([128, HID], BF16, tag=f"wkr{hc}", name=f"wkr{hc}")
            nc.sync.dma_start(t[:], wk[hc * 128:(hc + 1) * 128, :])
            wk_sb.append(t)
            t = res.tile([128, HID], BF16, tag=f"wqr{hc}", name=f"wqr{hc}")
            nc.sync.dma_start(t[:], wq[hc * 128:(hc + 1) * 128, :])
            wq_sb.append(t)

        # ---- V projection ---------------------------------------------------
        with tc.tile_pool(name="wvp", bufs=1) as wvp, \
             tc.tile_pool(name="psv", bufs=4, space="PSUM") as psv:
            for g in range(2):  # groups of 8 heads = 512 cols
                wv_g = []
                for hc in range(HC):
                    t = wvp.tile([128, 512], BF16, tag=f"wv{hc}")
                    nc.sync.dma_start(
                        t[:], wv[hc * 128:(hc + 1) * 128, g * 512:(g + 1) * 512])
                    wv_g.append(t)
                for sc in range(SC16):
                    ps = psv.tile([128, 512], F32, tag="psv")
                    for hc in range(HC):
                        nc.tensor.matmul(
                            ps[:], hsT_sb[hc][:, sc * 128:(sc + 1) * 128],
                            wv_g[hc][:], start=(hc == 0), stop=(hc == HC - 1))
                    for hh in range(8):
                        h = 8 * g + hh
                        nc.vector.tensor_copy(v_sb[sc][:, h * 65:h * 65 + 64],
                                              ps[:, hh * 64:(hh + 1) * 64])
                # ones columns for these heads
                for sc in range(SC16):
                    v3 = v_sb[sc][:].rearrange("p (h c) -> p h c", h=NH)
                    nc.gpsimd.tensor_copy(v3[:, 8 * g:8 * g + 8, 64],
                                          ones16[:, 8 * g:8 * g + 8])

        # ---- head-pair loop -------------------------------------------------
        with tc.tile_pool(name="wqk", bufs=2) as wqk, \
             tc.tile_pool(name="kt", bufs=2) as ktp, \
             tc.tile_pool(name="qt", bufs=2) as qtp, \
             tc.tile_pool(name="rope", bufs=2) as rope, \
             tc.tile_pool(name="expp", bufs=6) as expp, \
             tc.tile_pool(name="nrm", bufs=2) as nrm, \
             tc.tile_pool(name="psk", bufs=2, space="PSUM") as psk, \
             tc.tile_pool(name="pss", bufs=4, space="PSUM") as pss, \
             tc.tile_pool(name="psa", bufs=1, space="PSUM") as psa:

            def rope_apply(dst, ps, s0, n):
                """dst[:, s0:s0+n] = rope(ps) for a head pair [128, n]."""
                t1 = rope.tile([128, 512], F32, tag="t1")
                nc.vector.tensor_tensor(
                    t1[:, :n], ps[:, :n], cos_sb[:, s0:s0 + n], AluOpType.mult)
                t2 = rope.tile([128, 512], F32, tag="t2")
                for hb in (0, 64):
                    nc.vector.tensor_tensor(
                        t2[hb:hb + 32, :n], ps[hb + 32:hb + 64, :n],
                        sin_sb[hb:hb + 32, s0:s0 + n], AluOpType.mult)
                    nc.vector.tensor_tensor(
                        t2[hb + 32:hb + 64, :n], ps[hb:hb + 32, :n],
                        sin_sb[hb + 32:hb + 64, s0:s0 + n], AluOpType.mult)
                nc.vector.tensor_tensor(
                    dst[:, s0:s0 + n], t1[:, :n], t2[:, :n], AluOpType.add)

            for p in range(PAIRS):
                c0 = p * 128

                # K projection + RoPE -> kT pair-packed [128, S]
                kt_pr = ktp.tile([128, S], BF16, tag="kt")
                for sc in range(SC4):
                    ps = psk.tile([128, 512], F32, tag="psk")
                    for hc in range(HC):
                        nc.tensor.matmul(
                            ps[:], wk_sb[hc][:, c0:c0 + 128],
                            hsT_sb[hc][:, sc * 512:(sc + 1) * 512],
                            start=(hc == 0), stop=(hc == HC - 1))
                    rope_apply(kt_pr, ps, sc * 512, 512)

                # Q projection + RoPE -> qT pair-packed [128, Q]
                qt_pr = qtp.tile([128, Q], BF16, tag="qt")
                ps = psk.tile([128, 512], F32, tag="psk")
                for hc in range(HC):
                    nc.tensor.matmul(ps[:], wq_sb[hc][:, c0:c0 + 128],
                                     hsT_sb[hc][:, 0:Q],
                                     start=(hc == 0), stop=(hc == HC - 1))
                rope_apply(qt_pr, ps, 0, Q)

                if dbg and p == 0:
                    nc.sync.dma_start(d_kt[:], kt_pr[:].bitcast(F32))
                    nc.sync.dma_start(d_qt[:], qt_pr[:].bitcast(F32))

                # attention per head
                ps_a2 = [psa.tile([65, Q], F32, tag=f"psa{half}",
                                  name=f"psa{half}") for half in range(2)]
                for sc in range(SC16):
                    for half in range(2):
                        h = 2 * p + half
                        hb = half * 64
                        ps_s = pss.tile([128, Q], F32, tag="pss")
                        nc.tensor.matmul(
                            ps_s[:], kt_pr[hb:hb + 64, sc * 128:(sc + 1) * 128],
                            qt_pr[hb:hb + 64, :], start=True, stop=True)
                        t_exp = expp.tile([128, Q], F16, tag="texp")
                        nc.scalar.activation(t_exp[:], ps_s[:], AF.Exp)
                        t_em = expp.tile([128, Q], F16, tag="tem")
                        eng = nc.vector if (sc % 2 == 0) else nc.gpsimd
                        eng.tensor_tensor(t_em[:], t_exp[:], em_sb[sc][:],
                                          AluOpType.mult)
                        if dbg and p == 0 and half == 0 and sc == 0:
                            t_d = expp.tile([128, Q], F32, tag="tdbg")
                            nc.vector.tensor_copy(t_d[:], t_em[:])
                            nc.sync.dma_start(d_esm[:], t_d[:])
                        nc.tensor.matmul(ps_a2[half][:],
                                         v_sb[sc][:, h * 65:h * 65 + 65],
                                         t_em[:], start=(sc == 0),
                                         stop=(sc == SC16 - 1))
                for half in range(2):
                    hb = half * 64
                    ps_a = ps_a2[half]
                    # normalize by denominator row (64) and store to acc
                    if dbg and p == 0:
                        t_d2 = nrm.tile([65, Q], F32, tag="tdbg2")
                        nc.vector.tensor_copy(t_d2[:], ps_a[:])
                        nc.sync.dma_start(d_psa[half * 65:(half + 1) * 65, :],
                                          t_d2[:])
                    h = 2 * p + half
                    dtmp = nrm.tile([1, Q], F32, tag="dtmp")
                    nc.scalar.copy(dtmp[:], ps_a[64:65, :])
                    nc.sync.dma_start(den_all[h:h + 1, :], dtmp[:])
                    nc.vector.tensor_copy(acc_sb[p][hb:hb + 64, :],
                                          ps_a[0:64, :])

        if dbg:
            with tc.tile_pool(name="dbgp", bufs=1) as dbgp:
                t_d3 = dbgp.tile([128, Q], F32, tag="td3")
                nc.vector.tensor_copy(t_d3[:], acc_sb[0][:])
                nc.sync.dma_start(d_acc[:], t_d3[:])
                t_d4 = dbgp.tile([128, NH * 65], F32, tag="td4")
                nc.vector.tensor_copy(t_d4[:], v_sb[0][:])
                nc.sync.dma_start(d_v[:], t_d4[:])

        # ---- normalize (deferred) + output projection -----------------------
        with tc.tile_pool(name="wop", bufs=1) as wop, \
             tc.tile_pool(name="outp", bufs=3) as outp, \
             tc.tile_pool(name="nrm2", bufs=1) as nrm2, \
             tc.tile_pool(name="psb", bufs=2, space="PSUM") as psb, \
             tc.tile_pool(name="pso", bufs=2, space="PSUM") as pso:
            sel_sb = nrm2.tile([NH, HID], F32R, tag="sel")
            nc.sync.dma_start(sel_sb[:], sel[:])
            recip_all = nrm2.tile([NH, Q], F32R, tag="recip")
            with nc.allow_low_precision(reason="f32r reciprocal broadcast"):
                nc.vector.reciprocal(recip_all[:], den_all[:])
            acc2 = []
            for p in range(PAIRS):
                ps_bc = psb.tile([128, Q], F32, tag="psb")
                nc.tensor.matmul(ps_bc[:], sel_sb[:, p * 128:(p + 1) * 128],
                                 recip_all[:], start=True, stop=True)
                a2 = nrm2.tile([128, Q], F16, tag=f"acc2_{p}", name=f"acc2_{p}")
                with nc.allow_low_precision(reason="fp16 attention weights"):
                    nc.vector.tensor_tensor(a2[:], acc_sb[p][:], ps_bc[:],
                                            AluOpType.mult)
                acc2.append(a2)
            wo_p = []
            for p in range(PAIRS):
                t = wop.tile([128, HID], F16, tag=f"wo{p}")
                nc.sync.dma_start(t[:], wo[p * 128:(p + 1) * 128, :])
                wo_p.append(t)
            for qc in range(Q // 128):
                for nn in range(2):
                    ps = pso.tile([128, 512], F32, tag="pso")
                    for p in range(PAIRS):
                        nc.tensor.matmul(
                            ps[:], acc2[p][:, qc * 128:(qc + 1) * 128],
                            wo_p[p][:, nn * 512:(nn + 1) * 512],
                            start=(p == 0), stop=(p == PAIRS - 1))
                    t_out = outp.tile([128, 512], F32, tag="tout")
                    nc.vector.tensor_copy(t_out[:], ps[:])
                    nc.sync.dma_start(
                        out[qc * 128:(qc + 1) * 128, nn * 512:(nn + 1) * 512],
                        t_out[:])

    nc.compile()
    return nc


_NC_CACHE = None


def _get_program():
    global _NC_CACHE
    if _NC_CACHE is None:
        _NC_CACHE = build_program()
    return _NC_CACHE


def make_in_maps(hidden_states, attention_mask, position_ids, cos, sin,
                 Wq, Wk, Wv, Wo):
    import ml_dtypes
    bf16 = ml_dtypes.bfloat16
    hidden_states = np.asarray(hidden_states, np.float32)
    attention_mask = np.asarray(attention_mask, np.float32)
    position_ids = np.asarray(position_ids)
    cos = np.asarray(cos, np.float32)
    sin = np.asarray(sin, np.float32)
    wq_s = (np.asarray(Wq, np.float32) * SCALE).astype(bf16)
    wk_ = np.ascontiguousarray(np.asarray(Wk, np.float32)).astype(bf16)
    wv_ = np.ascontiguousarray(np.asarray(Wv, np.float32)).astype(bf16)
    wo_ = np.ascontiguousarray(np.asarray(Wo, np.float32)).astype(np.float16)

    sel = np.zeros((NH, HID), np.float32)
    for p in range(PAIRS):
        for m in range(128):
            sel[2 * p + (m >= 64), 128 * p + m] = 1.0

    in_maps = []
    for b in range(B):
        hsT_b = hidden_states[b].T  # [HID, S]
        cos_b = cos[position_ids[b]]  # [S, HD]
        sin_b = sin[position_ids[b]]
        cosT = np.tile(cos_b.T, (2, 1))  # [128, S] (two heads stacked)
        # sign-folded sin for rotate_half: rows 0:32 get -sin, 32:64 get +sin
        sinT = sin_b.T.copy()
        sinT[0:32] *= -1.0
        sinT = np.tile(sinT, (2, 1))  # [128, S]
        maskT_b = attention_mask[b, 0].T  # [S(keys), S(queries)]
        for qc in range(4):
            q0 = qc * Q
            roll = -q0
            in_maps.append({
                "hsT": np.ascontiguousarray(np.roll(hsT_b, roll, axis=1)).astype(bf16),
                "cosk": np.ascontiguousarray(np.roll(cosT, roll, axis=1)),
                "sink": np.ascontiguousarray(np.roll(sinT, roll, axis=1)),
                "emask": np.exp(
                    np.roll(maskT_b[:, q0:q0 + Q], roll, axis=0)
                ).astype(np.float16),
                "wq": wq_s, "wk": wk_, "wv": wv_, "wo": wo_, "sel": sel,
            })
    return in_maps


def run(inputs: dict, trace: bool = False):
    nc = _get_program()
    in_maps = make_in_maps(**inputs)
    res = run_bass_kernel_spmd(nc, in_maps, list(range(N_CORES)), trace=trace)
    out = np.empty((B, S, HID), np.float32)
    for c in range(N_CORES):
        b, qc = c // 4, c % 4
        out[b, qc * Q:(qc + 1) * Q, :] = res.results[c]["out"]
    return out, res


def kernel(**inputs) -> np.ndarray:
    out, _ = run(inputs, trace=False)
    return out
